# revision 8
# baseline (speedup 1.0000x reference)
"""Trainium2 Bass kernel for DSSConv2d (low-rank spatially-combined 3x3 conv).

Computation (per reference):
  convs = conv2d(x, w.reshape(rank*oc, ic, 3, 3), pad=1)   # [B, rank*oc, H, W]
  cw    = softmax(cw_row + cw_col, axis=0)                 # [rank, H, W]
  out   = einsum('bkcxy,kxy->bcxy', convs.reshape(B,rank,oc,H,W), cw)
  out  += b + b_col + b_row

Strategy:
  - Data parallel: batch 32 -> 4 images per core on 8 cores.
  - 3x3 conv = 9 shifted 1x1 convs: for each (dy,dx), a [ic,oc] matmul over a
    shifted slice of zero-padded x, accumulated in PSUM. bf16 inputs, fp32 acc.
  - Layout: psum[oc, pix] with weights stationary; pixel tiles = 8 image rows
    (N = 8*56 = 448 <= 512 PSUM bank).
  - Per-pixel rank combine on VectorE with the softmax weights broadcast
    across partitions via a stride-0 DMA; bias plane precomputed on host.
"""

import numpy as np
import ml_dtypes
from contextlib import ExitStack

import concourse.bass as bass
import concourse.mybir as mybir
import concourse.tile as tile
from concourse import bacc
from concourse.bass_utils import run_bass_kernel_spmd

RANK, OC, IC = 4, 128, 128
B, H, W = 32, 56, 56
NCORES = 8
B_LOC = B // NCORES          # 4 images per core
HP, WP = H + 2, W + 2        # zero-padded input: 58 x 58
PIX = H * W                  # 3136
RB = 8                       # output rows per pixel tile
NBLK = H // RB               # 7 tiles per image
NT = RB * W                  # 448 pixels per tile

BF16 = mybir.dt.bfloat16
F32 = mybir.dt.float32

_CACHE = {}


def _build_nc():
    nc = bacc.Bacc()
    x_in = nc.dram_tensor("x", [B_LOC, IC, HP, WP], BF16, kind="ExternalInput")
    w_in = nc.dram_tensor("w", [IC, RANK, 9, OC], BF16, kind="ExternalInput")
    cw_in = nc.dram_tensor("cw", [RANK, PIX], BF16, kind="ExternalInput")
    bias_in = nc.dram_tensor("bias", [OC, PIX], BF16, kind="ExternalInput")
    out = nc.dram_tensor("out", [B_LOC, OC, PIX], BF16, kind="ExternalOutput")

    with tile.TileContext(nc) as tc, ExitStack() as ctx:
        consts = ctx.enter_context(tc.tile_pool(name="consts", bufs=1))
        xpool = ctx.enter_context(tc.tile_pool(name="xpool", bufs=2))
        mpool = ctx.enter_context(tc.tile_pool(name="mpool", bufs=2))
        opool = ctx.enter_context(tc.tile_pool(name="opool", bufs=3))
        pspool = ctx.enter_context(tc.tile_pool(name="ps", bufs=8, space="PSUM"))

        # Row blocks per image: 8 rows -> matmul N = 448 (<=512 ISA limit).
        BLKS = [(h0, 8) for h0 in range(0, H, 8)]
        NTMAX = 8 * W

        # x in standalone row chunks per image (rows incl. 3x3 halo) so the
        # first matmuls wait on a ~0.26 MB transfer, not a full image.
        XCH = [(0, 18), (16, 18), (32, 18), (48, 10)]
        # image 0 split finer and spread across rings: first block only
        # needs padded rows [0, 10)
        XCH0 = [(0, 10), (8, 18), (24, 18), (40, 18)]

        RINGS = [nc.sync, nc.scalar, nc.gpsimd]

        def load_img(img, layout, spread=False):
            chunks = []
            for ci, (r0, nr) in enumerate(layout):
                xc = xpool.tile([IC, nr, WP], BF16, name=f"x{ci}", tag=f"x{ci}")
                eng = RINGS[ci % len(RINGS)] if spread else nc.scalar
                eng.dma_start(out=xc, in_=x_in[img][:, r0 : r0 + nr, :])
                chunks.append(xc)
            return chunks

        def chunk_for(layout, h0, nr):
            # deepest chunk that covers padded rows [h0, h0+nr+2)
            for ci in range(len(layout) - 1, -1, -1):
                r0, cnr = layout[ci]
                if r0 <= h0 and h0 + nr + 2 <= r0 + cnr:
                    return ci
            raise AssertionError((h0, nr))

        # Startup critical path. All DMAs are issued in first-use order and
        # round-robined across the three HWDGE rings so no ring develops a
        # deep queue ahead of a deadline:
        #   1. x image-0 chunk 0 (block 0's rhs) alone at the head of the
        #      scalar ring.
        #   2. every rank's weights per-tap (36 x 32 KB) -- tap (r, o) is
        #      consumed ~0.2 us apart, so round-robin delivery always stays
        #      ahead of the matmul stream (v2 stalled 11 us waiting for the
        #      monolithic 0.3 MB rank-1 weight load).
        #   3. remaining image-0 chunks.
        #   4. cw broadcasts + bias, chunked per row-block so block 0's
        #      combine waits on ~0.5 MB, not 4 MB.
        x0c0 = xpool.tile([IC, XCH0[0][1], WP], BF16, name="x0", tag="x0")
        nc.scalar.dma_start(out=x0c0, in_=x_in[0][:, 0 : XCH0[0][1], :])

        rr = [0]

        def ring():
            rr[0] += 1
            return RINGS[rr[0] % 3]

        w_sb = []
        for r in range(RANK):
            wr = consts.tile([IC, 9, OC], BF16, name=f"w{r}")
            w_sb.append(wr)
        for r in range(RANK):
            for o in range(9):
                ring().dma_start(out=w_sb[r][:, o, :], in_=w_in[:, r, o, :])

        x0 = [x0c0]
        for ci, (r0, nr) in enumerate(XCH0[1:], start=1):
            xc = xpool.tile([IC, nr, WP], BF16, name=f"x{ci}", tag=f"x{ci}")
            ring().dma_start(out=xc, in_=x_in[0][:, r0 : r0 + nr, :])
            x0.append(xc)

        # softmax combine weights (bf16) replicated across all 128
        # partitions, and the bias plane, both in per-block chunks issued
        # in the order the combines consume them
        cwb = [
            consts.tile([128, PIX], BF16, name=f"cw{r}") for r in range(RANK)
        ]
        bias_sb = consts.tile([OC, PIX], BF16)
        for blk in range(NBLK):
            p0, p1 = blk * NT, (blk + 1) * NT
            for r in range(RANK):
                ring().dma_start(
                    out=cwb[r][:, p0:p1],
                    in_=cw_in[r, p0:p1].partition_broadcast(128),
                )
            ring().dma_start(out=bias_sb[:, p0:p1], in_=bias_in[:, p0:p1])

        # HAM warmup: dummy matmuls from t~0.3us cover the HAM cold window
        # and the DMA preamble, so real matmuls start at full clock. The
        # warm tile comes from a DVE memset (fast; GPSIMD ucode boot used
        # to delay this by ~6us). A throwaway reader releases the PSUM
        # slot back to the pool before the main loop needs all 8 banks.
        warm = consts.tile([128, NTMAX], BF16, name="warm")
        nc.vector.memset(warm, 0.0)
        wps = pspool.tile([OC, NTMAX], F32, name="wps", tag="ps")
        for _ in range(10):
            nc.tensor.matmul(
                wps[:, :], lhsT=warm[:, :128], rhs=warm, start=True, stop=True
            )
        wsink = consts.tile([128, 1], F32, name="wsink")
        nc.vector.tensor_copy(wsink, wps[:, 0:1])

        x_cur, lay_cur = x0, XCH0
        for img in range(B_LOC):
            if img + 1 < B_LOC:
                x_nxt = load_img(img + 1, XCH)
            blks = BLKS
            if img == B_LOC - 1:
                # shorter final blocks -> shorter post-matmul DVE/DMA tail
                blks = BLKS[:-1] + [(48, 4), (52, 2), (54, 2)]
            for blk, (h0, nr) in enumerate(blks):
                ci = chunk_for(lay_cur, h0, nr)
                xc = x_cur[ci]
                hoff = h0 - lay_cur[ci][0]
                nt = nr * W
                p0 = h0 * W
                psums = []
                for r in range(RANK):
                    ps = pspool.tile(
                        [OC, NTMAX], F32, name=f"ps{r}", tag="ps"
                    )
                    for o in range(9):
                        dy, dx = o // 3, o % 3
                        nc.tensor.matmul(
                            ps[:, :nt],
                            lhsT=w_sb[r][:, o, :],
                            rhs=xc[:, hoff + dy : hoff + dy + nr, dx : dx + W],
                            start=(o == 0),
                            stop=(o == 8),
                        )
                    psums.append(ps)
                # combine: all four rank muls first (frees the PSUM banks
                # as early as possible), then a bf16 add tree (DVE 2x
                # packed mode) with the bias folded into the last add.
                ms = []
                for r in range(RANK):
                    m = mpool.tile([OC, NTMAX], BF16, name=f"m{r}", tag=f"m{r}")
                    nc.vector.tensor_mul(
                        m[:, :nt], psums[r][:, :nt], cwb[r][:, p0 : p0 + nt]
                    )
                    ms.append(m)
                t0 = mpool.tile([OC, NTMAX], BF16, name="t0", tag="t0")
                nc.vector.tensor_add(t0[:, :nt], ms[0][:, :nt], ms[1][:, :nt])
                t1 = mpool.tile([OC, NTMAX], BF16, name="t1", tag="t1")
                nc.vector.tensor_add(t1[:, :nt], ms[2][:, :nt], ms[3][:, :nt])
                t2 = mpool.tile([OC, NTMAX], BF16, name="t2", tag="t2")
                nc.vector.tensor_add(t2[:, :nt], t0[:, :nt], t1[:, :nt])
                ot = opool.tile([OC, NTMAX], BF16, name="ot", tag="ot")
                nc.vector.tensor_add(
                    ot[:, :nt], t2[:, :nt], bias_sb[:, p0 : p0 + nt]
                )
                RINGS[2 * (blk % 2)].dma_start(
                    out=out[img][:, p0 : p0 + nt], in_=ot[:, :nt]
                )
            if img + 1 < B_LOC:
                x_cur, lay_cur = x_nxt, XCH
    nc.finalize()
    return nc


def _prep_inputs(x, w, cw_row, cw_col, b_row, b_col, b):
    # zero-padded bf16 input
    xp = np.zeros((B, IC, HP, WP), dtype=ml_dtypes.bfloat16)
    xp[:, :, 1 : H + 1, 1 : W + 1] = x.astype(ml_dtypes.bfloat16)

    # weights: [rank, oc, ic, kh, kw] -> [ic, rank, kh*kw, oc], bf16
    wt = np.ascontiguousarray(np.transpose(w, (2, 0, 3, 4, 1))).reshape(
        IC, RANK, 9, OC
    ).astype(ml_dtypes.bfloat16)

    # softmax over rank of per-pixel combine logits
    logits = (cw_row + cw_col).astype(np.float64)  # [rank, H, W]
    logits -= logits.max(axis=0, keepdims=True)
    e = np.exp(logits)
    cw = (
        (e / e.sum(axis=0, keepdims=True))
        .astype(ml_dtypes.bfloat16)
        .reshape(RANK, PIX)
    )

    # combined bias plane [oc, pix]
    bias = (
        b.reshape(OC, 1, 1) + b_row.reshape(1, H, 1) + b_col.reshape(1, 1, W)
    ).astype(ml_dtypes.bfloat16).reshape(OC, PIX)

    return xp, wt, cw, bias


def _run(inputs, trace=False):
    if "nc" not in _CACHE:
        _CACHE["nc"] = _build_nc()
    nc = _CACHE["nc"]
    xp, wt, cw, bias = _prep_inputs(**inputs)
    in_maps = [
        {"x": xp[c * B_LOC : (c + 1) * B_LOC], "w": wt, "cw": cw, "bias": bias}
        for c in range(NCORES)
    ]
    res = run_bass_kernel_spmd(nc, in_maps, list(range(NCORES)), trace=trace)
    outs = [np.asarray(res.results[c]["out"]) for c in range(NCORES)]
    full = np.concatenate(outs, axis=0).reshape(B, OC, H, W).astype(np.float32)
    return full, res


def kernel(**inputs):
    full, _ = _run(inputs)
    return full



# revision 9
# speedup vs baseline: 1.1609x; 1.1609x over previous
"""Trainium2 Bass kernel for DSSConv2d (low-rank spatially-combined 3x3 conv).

Computation (per reference):
  convs = conv2d(x, w.reshape(rank*oc, ic, 3, 3), pad=1)   # [B, rank*oc, H, W]
  cw    = softmax(cw_row + cw_col, axis=0)                 # [rank, H, W]
  out   = einsum('bkcxy,kxy->bcxy', convs.reshape(B,rank,oc,H,W), cw)
  out  += b + b_col + b_row

Strategy:
  - Data parallel: batch 32 -> 4 images per core on 8 cores.
  - 3x3 conv = 9 shifted 1x1 convs: for each (dy,dx), a [ic,oc] matmul over a
    shifted slice of zero-padded x, accumulated in PSUM. bf16 inputs, fp32 acc.
  - Layout: psum[oc, pix] with weights stationary; pixel tiles = 8 image rows
    (N = 8*56 = 448 <= 512 PSUM bank).
  - Per-pixel rank combine on VectorE with the softmax weights broadcast
    across partitions via a stride-0 DMA; bias plane precomputed on host.
"""

import numpy as np
import ml_dtypes
from contextlib import ExitStack

import concourse.bass as bass
import concourse.mybir as mybir
import concourse.tile as tile
from concourse import bacc
from concourse.bass_utils import run_bass_kernel_spmd

RANK, OC, IC = 4, 128, 128
B, H, W = 32, 56, 56
NCORES = 8
B_LOC = B // NCORES          # 4 images per core
HP, WP = H + 2, W + 2        # zero-padded input: 58 x 58
PIX = H * W                  # 3136
RB = 8                       # output rows per pixel tile
NBLK = H // RB               # 7 tiles per image
NT = RB * W                  # 448 pixels per tile

BF16 = mybir.dt.bfloat16
F32 = mybir.dt.float32

_CACHE = {}


def _build_nc():
    nc = bacc.Bacc()
    x_in = nc.dram_tensor("x", [B_LOC, IC, HP, WP], BF16, kind="ExternalInput")
    w_in = nc.dram_tensor("w", [IC, RANK, 9, OC], BF16, kind="ExternalInput")
    cw_in = nc.dram_tensor("cw", [RANK, PIX], BF16, kind="ExternalInput")
    bias_in = nc.dram_tensor("bias", [OC, PIX], BF16, kind="ExternalInput")
    out = nc.dram_tensor("out", [B_LOC, OC, PIX], BF16, kind="ExternalOutput")

    with tile.TileContext(nc) as tc, ExitStack() as ctx:
        consts = ctx.enter_context(tc.tile_pool(name="consts", bufs=1))
        xpool = ctx.enter_context(tc.tile_pool(name="xpool", bufs=2))
        mpool = ctx.enter_context(tc.tile_pool(name="mpool", bufs=2))
        opool = ctx.enter_context(tc.tile_pool(name="opool", bufs=3))
        pspool = ctx.enter_context(tc.tile_pool(name="ps", bufs=8, space="PSUM"))

        # Row blocks per image: 8 rows -> matmul N = 448 (<=512 ISA limit).
        BLKS = [(h0, 8) for h0 in range(0, H, 8)]
        NTMAX = 8 * W

        # x in standalone row chunks per image (rows incl. 3x3 halo) so the
        # first matmuls wait on a ~0.26 MB transfer, not a full image.
        XCH = [(0, 18), (16, 18), (32, 18), (48, 10)]
        # image 0 split finer and spread across rings: first block only
        # needs padded rows [0, 10)
        XCH0 = [(0, 10), (8, 18), (24, 18), (40, 18)]

        RINGS = [nc.sync, nc.scalar, nc.gpsimd]

        def load_img(img, layout, spread=False):
            chunks = []
            for ci, (r0, nr) in enumerate(layout):
                xc = xpool.tile([IC, nr, WP], BF16, name=f"x{ci}", tag=f"x{ci}")
                eng = RINGS[ci % len(RINGS)] if spread else nc.scalar
                eng.dma_start(out=xc, in_=x_in[img][:, r0 : r0 + nr, :])
                chunks.append(xc)
            return chunks

        def chunk_for(layout, h0, nr):
            # deepest chunk that covers padded rows [h0, h0+nr+2)
            for ci in range(len(layout) - 1, -1, -1):
                r0, cnr = layout[ci]
                if r0 <= h0 and h0 + nr + 2 <= r0 + cnr:
                    return ci
            raise AssertionError((h0, nr))

        # Startup critical path. All DMAs are issued in first-use order and
        # round-robined across the three HWDGE rings so no ring develops a
        # deep queue ahead of a deadline:
        #   1. x image-0 chunk 0 (block 0's rhs) alone at the head of the
        #      scalar ring.
        #   2. every rank's weights per-tap (36 x 32 KB) -- tap (r, o) is
        #      consumed ~0.2 us apart, so round-robin delivery always stays
        #      ahead of the matmul stream (v2 stalled 11 us waiting for the
        #      monolithic 0.3 MB rank-1 weight load).
        #   3. remaining image-0 chunks.
        #   4. cw broadcasts + bias, chunked per row-block so block 0's
        #      combine waits on ~0.5 MB, not 4 MB.
        x0c0 = xpool.tile([IC, XCH0[0][1], WP], BF16, name="x0", tag="x0")
        nc.scalar.dma_start(out=x0c0, in_=x_in[0][:, 0 : XCH0[0][1], :])

        rr = [0]

        def ring():
            rr[0] += 1
            return RINGS[rr[0] % 3]

        w_sb = []
        for r in range(RANK):
            wr = consts.tile([IC, 9, OC], BF16, name=f"w{r}")
            w_sb.append(wr)
        for r in range(RANK):
            for o in range(9):
                ring().dma_start(out=w_sb[r][:, o, :], in_=w_in[:, r, o, :])

        x0 = [x0c0]
        for ci, (r0, nr) in enumerate(XCH0[1:], start=1):
            xc = xpool.tile([IC, nr, WP], BF16, name=f"x{ci}", tag=f"x{ci}")
            ring().dma_start(out=xc, in_=x_in[0][:, r0 : r0 + nr, :])
            x0.append(xc)

        # softmax combine weights (bf16) replicated across all 128
        # partitions, and the bias plane, both in per-block chunks issued
        # in the order the combines consume them
        cwb = [
            consts.tile([128, PIX], BF16, name=f"cw{r}") for r in range(RANK)
        ]
        bias_sb = consts.tile([OC, PIX], BF16)
        for blk in range(NBLK):
            p0, p1 = blk * NT, (blk + 1) * NT
            for r in range(RANK):
                ring().dma_start(
                    out=cwb[r][:, p0:p1],
                    in_=cw_in[r, p0:p1].partition_broadcast(128),
                )
            ring().dma_start(out=bias_sb[:, p0:p1], in_=bias_in[:, p0:p1])

        # HAM warmup: dummy matmuls from t~0.3us cover the HAM cold window
        # and the DMA preamble, so real matmuls start at full clock. The
        # warm tile comes from a DVE memset (fast; GPSIMD ucode boot used
        # to delay this by ~6us). A throwaway reader releases the PSUM
        # slot back to the pool before the main loop needs all 8 banks.
        warm = consts.tile([128, NTMAX], BF16, name="warm")
        nc.vector.memset(warm, 0.0)
        wps = pspool.tile([OC, NTMAX], F32, name="wps", tag="ps")
        for _ in range(10):
            nc.tensor.matmul(
                wps[:, :], lhsT=warm[:, :128], rhs=warm, start=True, stop=True
            )
        wsink = consts.tile([128, 1], F32, name="wsink")
        nc.vector.tensor_copy(wsink, wps[:, 0:1])

        x_cur, lay_cur = x0, XCH0
        for img in range(B_LOC):
            if img + 1 < B_LOC:
                x_nxt = load_img(img + 1, XCH)
            blks = BLKS
            if img == B_LOC - 1:
                # shorter final blocks -> shorter post-matmul DVE/DMA tail
                blks = BLKS[:-1] + [(48, 4), (52, 2), (54, 2)]
            # Pair full-height row blocks: two matmuls share each stationary
            # weight load (consecutive same-lhsT matmuls), halving the
            # per-matmul LDWEIGHTS dispatch overhead on the PE sequencer.
            groups = []
            i = 0
            while i < len(blks):
                if i + 1 < len(blks) and blks[i][1] == 8 and blks[i + 1][1] == 8:
                    groups.append([blks[i], blks[i + 1]])
                    i += 2
                else:
                    groups.append([blks[i]])
                    i += 1
            for gi, grp in enumerate(groups):
                infos = []
                for h0, nr in grp:
                    ci = chunk_for(lay_cur, h0, nr)
                    infos.append(
                        (h0, nr, nr * W, h0 * W, x_cur[ci], h0 - lay_cur[ci][0])
                    )
                psums = []  # [rank][block-in-group]
                for r in range(RANK):
                    pss = [
                        pspool.tile([OC, NTMAX], F32, name=f"ps{r}", tag="ps")
                        for _ in grp
                    ]
                    for o in range(9):
                        dy, dx = o // 3, o % 3
                        for bi, (h0, nr, nt, p0, xc, hoff) in enumerate(infos):
                            nc.tensor.matmul(
                                pss[bi][:, :nt],
                                lhsT=w_sb[r][:, o, :],
                                rhs=xc[
                                    :, hoff + dy : hoff + dy + nr, dx : dx + W
                                ],
                                start=(o == 0),
                                stop=(o == 8),
                            )
                    psums.append(pss)
                # combine: all rank muls first (frees the PSUM banks as
                # early as possible), then a bf16 add tree (DVE 2x packed
                # mode) with the bias folded into the last add.
                for bi, (h0, nr, nt, p0, xc, hoff) in enumerate(infos):
                    ms = []
                    for r in range(RANK):
                        m = mpool.tile(
                            [OC, NTMAX], BF16, name=f"m{r}", tag=f"m{r}"
                        )
                        nc.vector.tensor_mul(
                            m[:, :nt],
                            psums[r][bi][:, :nt],
                            cwb[r][:, p0 : p0 + nt],
                        )
                        ms.append(m)
                    t0 = mpool.tile([OC, NTMAX], BF16, name="t0", tag="t0")
                    nc.vector.tensor_add(
                        t0[:, :nt], ms[0][:, :nt], ms[1][:, :nt]
                    )
                    t1 = mpool.tile([OC, NTMAX], BF16, name="t1", tag="t1")
                    nc.vector.tensor_add(
                        t1[:, :nt], ms[2][:, :nt], ms[3][:, :nt]
                    )
                    t2 = mpool.tile([OC, NTMAX], BF16, name="t2", tag="t2")
                    nc.vector.tensor_add(t2[:, :nt], t0[:, :nt], t1[:, :nt])
                    ot = opool.tile([OC, NTMAX], BF16, name="ot", tag="ot")
                    nc.vector.tensor_add(
                        ot[:, :nt], t2[:, :nt], bias_sb[:, p0 : p0 + nt]
                    )
                    RINGS[2 * ((gi + bi) % 2)].dma_start(
                        out=out[img][:, p0 : p0 + nt], in_=ot[:, :nt]
                    )
            if img + 1 < B_LOC:
                x_cur, lay_cur = x_nxt, XCH
    nc.finalize()
    return nc


def _prep_inputs(x, w, cw_row, cw_col, b_row, b_col, b):
    # zero-padded bf16 input
    xp = np.zeros((B, IC, HP, WP), dtype=ml_dtypes.bfloat16)
    xp[:, :, 1 : H + 1, 1 : W + 1] = x.astype(ml_dtypes.bfloat16)

    # weights: [rank, oc, ic, kh, kw] -> [ic, rank, kh*kw, oc], bf16
    wt = np.ascontiguousarray(np.transpose(w, (2, 0, 3, 4, 1))).reshape(
        IC, RANK, 9, OC
    ).astype(ml_dtypes.bfloat16)

    # softmax over rank of per-pixel combine logits
    logits = (cw_row + cw_col).astype(np.float64)  # [rank, H, W]
    logits -= logits.max(axis=0, keepdims=True)
    e = np.exp(logits)
    cw = (
        (e / e.sum(axis=0, keepdims=True))
        .astype(ml_dtypes.bfloat16)
        .reshape(RANK, PIX)
    )

    # combined bias plane [oc, pix]
    bias = (
        b.reshape(OC, 1, 1) + b_row.reshape(1, H, 1) + b_col.reshape(1, 1, W)
    ).astype(ml_dtypes.bfloat16).reshape(OC, PIX)

    return xp, wt, cw, bias


def _run(inputs, trace=False):
    if "nc" not in _CACHE:
        _CACHE["nc"] = _build_nc()
    nc = _CACHE["nc"]
    xp, wt, cw, bias = _prep_inputs(**inputs)
    in_maps = [
        {"x": xp[c * B_LOC : (c + 1) * B_LOC], "w": wt, "cw": cw, "bias": bias}
        for c in range(NCORES)
    ]
    res = run_bass_kernel_spmd(nc, in_maps, list(range(NCORES)), trace=trace)
    outs = [np.asarray(res.results[c]["out"]) for c in range(NCORES)]
    full = np.concatenate(outs, axis=0).reshape(B, OC, H, W).astype(np.float32)
    return full, res


def kernel(**inputs):
    full, _ = _run(inputs)
    return full



# revision 10
# speedup vs baseline: 1.1909x; 1.0259x over previous
"""Trainium2 Bass kernel for DSSConv2d (low-rank spatially-combined 3x3 conv).

Computation (per reference):
  convs = conv2d(x, w.reshape(rank*oc, ic, 3, 3), pad=1)   # [B, rank*oc, H, W]
  cw    = softmax(cw_row + cw_col, axis=0)                 # [rank, H, W]
  out   = einsum('bkcxy,kxy->bcxy', convs.reshape(B,rank,oc,H,W), cw)
  out  += b + b_col + b_row

Strategy:
  - Data parallel: batch 32 -> 4 images per core on 8 cores.
  - 3x3 conv = 9 shifted 1x1 convs: for each (dy,dx), a [ic,oc] matmul over a
    shifted slice of zero-padded x, accumulated in PSUM. bf16 inputs, fp32 acc.
  - Layout: psum[oc, pix] with weights stationary; pixel tiles = 8 image rows
    (N = 8*56 = 448 <= 512 PSUM bank).
  - Per-pixel rank combine on VectorE with the softmax weights broadcast
    across partitions via a stride-0 DMA; bias plane precomputed on host.
"""

import numpy as np
import ml_dtypes
from contextlib import ExitStack

import concourse.bass as bass
import concourse.mybir as mybir
import concourse.tile as tile
from concourse import bacc
from concourse.bass_utils import run_bass_kernel_spmd

RANK, OC, IC = 4, 128, 128
B, H, W = 32, 56, 56
NCORES = 8
B_LOC = B // NCORES          # 4 images per core
HP, WP = H + 2, W + 2        # zero-padded input: 58 x 58
PIX = H * W                  # 3136
RB = 8                       # output rows per pixel tile
NBLK = H // RB               # 7 tiles per image
NT = RB * W                  # 448 pixels per tile

BF16 = mybir.dt.bfloat16
F32 = mybir.dt.float32

_CACHE = {}


def _build_nc():
    nc = bacc.Bacc()
    x_in = nc.dram_tensor("x", [B_LOC, IC, HP, WP], BF16, kind="ExternalInput")
    w_in = nc.dram_tensor("w", [IC, RANK, 9, OC], BF16, kind="ExternalInput")
    cw_in = nc.dram_tensor("cw", [RANK, PIX], BF16, kind="ExternalInput")
    bias_in = nc.dram_tensor("bias", [OC, PIX], BF16, kind="ExternalInput")
    out = nc.dram_tensor("out", [B_LOC, OC, PIX], BF16, kind="ExternalOutput")

    with tile.TileContext(nc) as tc, ExitStack() as ctx:
        consts = ctx.enter_context(tc.tile_pool(name="consts", bufs=1))
        xpool = ctx.enter_context(tc.tile_pool(name="xpool", bufs=2))
        mpool = ctx.enter_context(tc.tile_pool(name="mpool", bufs=2))
        opool = ctx.enter_context(tc.tile_pool(name="opool", bufs=3))
        pspool = ctx.enter_context(tc.tile_pool(name="ps", bufs=8, space="PSUM"))

        # Row blocks per image: 8 rows -> matmul N = 448 (<=512 ISA limit).
        BLKS = [(h0, 8) for h0 in range(0, H, 8)]
        NTMAX = 8 * W

        # x in standalone row chunks per image (rows incl. 3x3 halo) so the
        # first matmuls wait on a ~0.26 MB transfer, not a full image.
        XCH = [(0, 18), (16, 18), (32, 18), (48, 10)]
        # image 0 split finer and spread across rings: first block only
        # needs padded rows [0, 10)
        XCH0 = [(0, 10), (8, 18), (24, 18), (40, 18)]

        RINGS = [nc.sync, nc.scalar, nc.gpsimd]

        def load_img(img, layout, spread=False):
            chunks = []
            for ci, (r0, nr) in enumerate(layout):
                xc = xpool.tile([IC, nr, WP], BF16, name=f"x{ci}", tag=f"x{ci}")
                eng = RINGS[ci % len(RINGS)] if spread else nc.scalar
                eng.dma_start(out=xc, in_=x_in[img][:, r0 : r0 + nr, :])
                chunks.append(xc)
            return chunks

        def chunk_for(layout, h0, nr):
            # deepest chunk that covers padded rows [h0, h0+nr+2)
            for ci in range(len(layout) - 1, -1, -1):
                r0, cnr = layout[ci]
                if r0 <= h0 and h0 + nr + 2 <= r0 + cnr:
                    return ci
            raise AssertionError((h0, nr))

        # Startup critical path. All DMAs are issued in first-use order and
        # round-robined across the three HWDGE rings so no ring develops a
        # deep queue ahead of a deadline:
        #   1. x image-0 chunk 0 (block 0's rhs) alone at the head of the
        #      scalar ring.
        #   2. every rank's weights per-tap (36 x 32 KB) -- tap (r, o) is
        #      consumed ~0.2 us apart, so round-robin delivery always stays
        #      ahead of the matmul stream (v2 stalled 11 us waiting for the
        #      monolithic 0.3 MB rank-1 weight load).
        #   3. remaining image-0 chunks.
        #   4. cw broadcasts + bias, chunked per row-block so block 0's
        #      combine waits on ~0.5 MB, not 4 MB.
        x0c0 = xpool.tile([IC, XCH0[0][1], WP], BF16, name="x0", tag="x0")
        nc.scalar.dma_start(out=x0c0, in_=x_in[0][:, 0 : XCH0[0][1], :])

        rr = [0]

        def ring():
            rr[0] += 1
            return RINGS[rr[0] % 3]

        w_sb = []
        for r in range(RANK):
            wr = consts.tile([IC, 9, OC], BF16, name=f"w{r}")
            w_sb.append(wr)
        for r in range(RANK):
            for o in range(9):
                ring().dma_start(out=w_sb[r][:, o, :], in_=w_in[:, r, o, :])

        x0 = [x0c0]
        for ci, (r0, nr) in enumerate(XCH0[1:], start=1):
            xc = xpool.tile([IC, nr, WP], BF16, name=f"x{ci}", tag=f"x{ci}")
            ring().dma_start(out=xc, in_=x_in[0][:, r0 : r0 + nr, :])
            x0.append(xc)

        # softmax combine weights (bf16) replicated across all 128
        # partitions, and the bias plane, both in per-block chunks issued
        # in the order the combines consume them
        cwb = [
            consts.tile([128, PIX], BF16, name=f"cw{r}") for r in range(RANK)
        ]
        bias_sb = consts.tile([OC, PIX], BF16)
        for blk in range(NBLK):
            p0, p1 = blk * NT, (blk + 1) * NT
            for r in range(RANK):
                ring().dma_start(
                    out=cwb[r][:, p0:p1],
                    in_=cw_in[r, p0:p1].partition_broadcast(128),
                )
            ring().dma_start(out=bias_sb[:, p0:p1], in_=bias_in[:, p0:p1])

        # HAM warmup: dummy matmuls from t~0.3us cover the HAM cold window
        # and the DMA preamble, so real matmuls start at full clock. The
        # warm tile comes from a DVE memset (fast; GPSIMD ucode boot used
        # to delay this by ~6us). A throwaway reader releases the PSUM
        # slot back to the pool before the main loop needs all 8 banks.
        warm = consts.tile([128, NTMAX], BF16, name="warm")
        nc.vector.memset(warm, 0.0)
        wps = pspool.tile([OC, NTMAX], F32, name="wps", tag="ps")
        for _ in range(10):
            nc.tensor.matmul(
                wps[:, :], lhsT=warm[:, :128], rhs=warm, start=True, stop=True
            )
        wsink = consts.tile([128, 1], F32, name="wsink")
        nc.vector.tensor_copy(wsink, wps[:, 0:1])

        x_cur, lay_cur = x0, XCH0
        for img in range(B_LOC):
            if img + 1 < B_LOC:
                x_nxt = load_img(img + 1, XCH)
            blks = BLKS
            if img == B_LOC - 1:
                # shorter final blocks -> shorter post-matmul DVE/DMA tail
                blks = BLKS[:-1] + [(48, 4), (52, 2), (54, 2)]
            for blk, (h0, nr) in enumerate(blks):
                ci = chunk_for(lay_cur, h0, nr)
                xc = x_cur[ci]
                hoff = h0 - lay_cur[ci][0]
                nt = nr * W
                p0 = h0 * W
                psums = []
                for r in range(RANK):
                    ps = pspool.tile([OC, NTMAX], F32, name=f"ps{r}", tag="ps")
                    for o in range(9):
                        dy, dx = o // 3, o % 3
                        nc.tensor.matmul(
                            ps[:, :nt],
                            lhsT=w_sb[r][:, o, :],
                            rhs=xc[:, hoff + dy : hoff + dy + nr, dx : dx + W],
                            start=(o == 0),
                            stop=(o == 8),
                        )
                    psums.append(ps)
                # combine: all four rank muls first (frees the PSUM banks
                # as early as possible), then a bf16 add tree (DVE 2x
                # packed mode) with the bias folded into the last add.
                ms = []
                for r in range(RANK):
                    m = mpool.tile([OC, NTMAX], BF16, name=f"m{r}", tag=f"m{r}")
                    nc.vector.tensor_mul(
                        m[:, :nt], psums[r][:, :nt], cwb[r][:, p0 : p0 + nt]
                    )
                    ms.append(m)
                t0 = mpool.tile([OC, NTMAX], BF16, name="t0", tag="t0")
                nc.vector.tensor_add(t0[:, :nt], ms[0][:, :nt], ms[1][:, :nt])
                t1 = mpool.tile([OC, NTMAX], BF16, name="t1", tag="t1")
                nc.vector.tensor_add(t1[:, :nt], ms[2][:, :nt], ms[3][:, :nt])
                t2 = mpool.tile([OC, NTMAX], BF16, name="t2", tag="t2")
                nc.vector.tensor_add(t2[:, :nt], t0[:, :nt], t1[:, :nt])
                ot = opool.tile([OC, NTMAX], BF16, name="ot", tag="ot")
                nc.vector.tensor_add(
                    ot[:, :nt], t2[:, :nt], bias_sb[:, p0 : p0 + nt]
                )
                RINGS[2 * (blk % 2)].dma_start(
                    out=out[img][:, p0 : p0 + nt], in_=ot[:, :nt]
                )
            if img + 1 < B_LOC:
                x_cur, lay_cur = x_nxt, XCH
    nc.finalize()
    return nc


def _prep_inputs(x, w, cw_row, cw_col, b_row, b_col, b):
    # zero-padded bf16 input
    xp = np.zeros((B, IC, HP, WP), dtype=ml_dtypes.bfloat16)
    xp[:, :, 1 : H + 1, 1 : W + 1] = x.astype(ml_dtypes.bfloat16)

    # weights: [rank, oc, ic, kh, kw] -> [ic, rank, kh*kw, oc], bf16
    wt = np.ascontiguousarray(np.transpose(w, (2, 0, 3, 4, 1))).reshape(
        IC, RANK, 9, OC
    ).astype(ml_dtypes.bfloat16)

    # softmax over rank of per-pixel combine logits
    logits = (cw_row + cw_col).astype(np.float64)  # [rank, H, W]
    logits -= logits.max(axis=0, keepdims=True)
    e = np.exp(logits)
    cw = (
        (e / e.sum(axis=0, keepdims=True))
        .astype(ml_dtypes.bfloat16)
        .reshape(RANK, PIX)
    )

    # combined bias plane [oc, pix]
    bias = (
        b.reshape(OC, 1, 1) + b_row.reshape(1, H, 1) + b_col.reshape(1, 1, W)
    ).astype(ml_dtypes.bfloat16).reshape(OC, PIX)

    return xp, wt, cw, bias


def _run(inputs, trace=False):
    if "nc" not in _CACHE:
        _CACHE["nc"] = _build_nc()
    nc = _CACHE["nc"]
    xp, wt, cw, bias = _prep_inputs(**inputs)
    in_maps = [
        {"x": xp[c * B_LOC : (c + 1) * B_LOC], "w": wt, "cw": cw, "bias": bias}
        for c in range(NCORES)
    ]
    res = run_bass_kernel_spmd(nc, in_maps, list(range(NCORES)), trace=trace)
    outs = [np.asarray(res.results[c]["out"]) for c in range(NCORES)]
    full = np.concatenate(outs, axis=0).reshape(B, OC, H, W).astype(np.float32)
    return full, res


def kernel(**inputs):
    full, _ = _run(inputs)
    return full



# revision 11
# speedup vs baseline: 1.2047x; 1.0115x over previous
"""Trainium2 Bass kernel for DSSConv2d (low-rank spatially-combined 3x3 conv).

Computation (per reference):
  convs = conv2d(x, w.reshape(rank*oc, ic, 3, 3), pad=1)   # [B, rank*oc, H, W]
  cw    = softmax(cw_row + cw_col, axis=0)                 # [rank, H, W]
  out   = einsum('bkcxy,kxy->bcxy', convs.reshape(B,rank,oc,H,W), cw)
  out  += b + b_col + b_row

Strategy:
  - Data parallel: batch 32 -> 4 images per core on 8 cores.
  - 3x3 conv = 9 shifted 1x1 convs: for each (dy,dx), a [ic,oc] matmul over a
    shifted slice of zero-padded x, accumulated in PSUM. bf16 inputs, fp32 acc.
  - Layout: psum[oc, pix] with weights stationary; pixel tiles = 8 image rows
    (N = 8*56 = 448 <= 512 PSUM bank).
  - Per-pixel rank combine on VectorE with the softmax weights broadcast
    across partitions via a stride-0 DMA; bias plane precomputed on host.
"""

import numpy as np
import ml_dtypes
from contextlib import ExitStack

import concourse.bass as bass
import concourse.mybir as mybir
import concourse.tile as tile
from concourse import bacc
from concourse.bass_utils import run_bass_kernel_spmd

RANK, OC, IC = 4, 128, 128
B, H, W = 32, 56, 56
NCORES = 8
B_LOC = B // NCORES          # 4 images per core
HP, WP = H + 2, W + 2        # zero-padded input: 58 x 58
PIX = H * W                  # 3136
RB = 8                       # output rows per pixel tile
NBLK = H // RB               # 7 tiles per image
NT = RB * W                  # 448 pixels per tile

BF16 = mybir.dt.bfloat16
F32 = mybir.dt.float32

_CACHE = {}


def _build_nc():
    nc = bacc.Bacc()
    x_in = nc.dram_tensor("x", [B_LOC, IC, HP, WP], BF16, kind="ExternalInput")
    w_in = nc.dram_tensor("w", [IC, RANK, 9, OC], BF16, kind="ExternalInput")
    cw_in = nc.dram_tensor("cw", [RANK, PIX], BF16, kind="ExternalInput")
    bias_in = nc.dram_tensor("bias", [OC, PIX], BF16, kind="ExternalInput")
    out = nc.dram_tensor("out", [B_LOC, OC, PIX], BF16, kind="ExternalOutput")

    with tile.TileContext(nc) as tc, ExitStack() as ctx:
        consts = ctx.enter_context(tc.tile_pool(name="consts", bufs=1))
        xpool = ctx.enter_context(tc.tile_pool(name="xpool", bufs=2))
        mpool = ctx.enter_context(tc.tile_pool(name="mpool", bufs=2))
        opool = ctx.enter_context(tc.tile_pool(name="opool", bufs=3))
        pspool = ctx.enter_context(tc.tile_pool(name="ps", bufs=8, space="PSUM"))

        # Row blocks per image: 8 rows -> matmul N = 448 (<=512 ISA limit).
        BLKS = [(h0, 8) for h0 in range(0, H, 8)]
        NTMAX = 8 * W

        # x in standalone row chunks per image (rows incl. 3x3 halo) so the
        # first matmuls wait on a ~0.26 MB transfer, not a full image.
        XCH = [(0, 18), (16, 18), (32, 18), (48, 10)]
        # image 0 split finer and spread across rings: first block only
        # needs padded rows [0, 10)
        XCH0 = [(0, 10), (8, 18), (24, 18), (40, 18)]

        RINGS = [nc.sync, nc.scalar, nc.gpsimd]

        def load_img(img, layout, spread=False):
            chunks = []
            for ci, (r0, nr) in enumerate(layout):
                xc = xpool.tile([IC, nr, WP], BF16, name=f"x{ci}", tag=f"x{ci}")
                eng = RINGS[ci % len(RINGS)] if spread else nc.scalar
                eng.dma_start(out=xc, in_=x_in[img][:, r0 : r0 + nr, :])
                chunks.append(xc)
            return chunks

        def chunk_for(layout, h0, nr):
            # deepest chunk that covers padded rows [h0, h0+nr+2)
            for ci in range(len(layout) - 1, -1, -1):
                r0, cnr = layout[ci]
                if r0 <= h0 and h0 + nr + 2 <= r0 + cnr:
                    return ci
            raise AssertionError((h0, nr))

        # Startup critical path. All DMAs are issued in first-use order and
        # round-robined across the three HWDGE rings so no ring develops a
        # deep queue ahead of a deadline:
        #   1. x image-0 chunk 0 (block 0's rhs) alone at the head of the
        #      scalar ring.
        #   2. every rank's weights per-tap (36 x 32 KB) -- tap (r, o) is
        #      consumed ~0.2 us apart, so round-robin delivery always stays
        #      ahead of the matmul stream (v2 stalled 11 us waiting for the
        #      monolithic 0.3 MB rank-1 weight load).
        #   3. remaining image-0 chunks.
        #   4. cw broadcasts + bias, chunked per row-block so block 0's
        #      combine waits on ~0.5 MB, not 4 MB.
        # Weights monolithic per rank on the sync ring: each slice reads
        # 2304 B contiguous per partition (good DMA line efficiency --
        # per-tap 256 B splits measured ~4x slower and stalled the PE), and
        # rank r always lands before its first matmul. x image 0 has the
        # scalar ring to itself; cw/bias go per-block on the gpsimd ring so
        # block 0's combine waits on ~0.5 MB, not 4 MB.
        x0 = load_img(0, XCH0)

        w_sb = []
        for r in range(RANK):
            wr = consts.tile([IC, 9, OC], BF16, name=f"w{r}")
            nc.sync.dma_start(out=wr, in_=w_in[:, r, :, :])
            w_sb.append(wr)

        cwb = [
            consts.tile([128, PIX], BF16, name=f"cw{r}") for r in range(RANK)
        ]
        bias_sb = consts.tile([OC, PIX], BF16)
        for blk in range(NBLK):
            p0, p1 = blk * NT, (blk + 1) * NT
            for r in range(RANK):
                nc.gpsimd.dma_start(
                    out=cwb[r][:, p0:p1],
                    in_=cw_in[r, p0:p1].partition_broadcast(128),
                )
            nc.gpsimd.dma_start(out=bias_sb[:, p0:p1], in_=bias_in[:, p0:p1])

        # HAM warmup: dummy matmuls from t~0.3us cover the HAM cold window
        # and the DMA preamble, so real matmuls start at full clock. The
        # warm tile comes from a DVE memset (fast; GPSIMD ucode boot used
        # to delay this by ~6us). A throwaway reader releases the PSUM
        # slot back to the pool before the main loop needs all 8 banks.
        warm = consts.tile([128, NTMAX], BF16, name="warm")
        nc.vector.memset(warm, 0.0)
        wps = pspool.tile([OC, NTMAX], F32, name="wps", tag="ps")
        for _ in range(10):
            nc.tensor.matmul(
                wps[:, :], lhsT=warm[:, :128], rhs=warm, start=True, stop=True
            )
        wsink = consts.tile([128, 1], F32, name="wsink")
        nc.vector.tensor_copy(wsink, wps[:, 0:1])

        x_cur, lay_cur = x0, XCH0
        for img in range(B_LOC):
            if img + 1 < B_LOC:
                x_nxt = load_img(img + 1, XCH)
            blks = BLKS
            if img == B_LOC - 1:
                # shorter final blocks -> shorter post-matmul DVE/DMA tail
                blks = BLKS[:-1] + [(48, 4), (52, 2), (54, 2)]
            for blk, (h0, nr) in enumerate(blks):
                ci = chunk_for(lay_cur, h0, nr)
                xc = x_cur[ci]
                hoff = h0 - lay_cur[ci][0]
                nt = nr * W
                p0 = h0 * W
                psums = []
                for r in range(RANK):
                    ps = pspool.tile([OC, NTMAX], F32, name=f"ps{r}", tag="ps")
                    for o in range(9):
                        dy, dx = o // 3, o % 3
                        nc.tensor.matmul(
                            ps[:, :nt],
                            lhsT=w_sb[r][:, o, :],
                            rhs=xc[:, hoff + dy : hoff + dy + nr, dx : dx + W],
                            start=(o == 0),
                            stop=(o == 8),
                        )
                    psums.append(ps)
                # combine: all four rank muls first (frees the PSUM banks
                # as early as possible), then a bf16 add tree (DVE 2x
                # packed mode) with the bias folded into the last add.
                ms = []
                for r in range(RANK):
                    m = mpool.tile([OC, NTMAX], BF16, name=f"m{r}", tag=f"m{r}")
                    nc.vector.tensor_mul(
                        m[:, :nt], psums[r][:, :nt], cwb[r][:, p0 : p0 + nt]
                    )
                    ms.append(m)
                t0 = mpool.tile([OC, NTMAX], BF16, name="t0", tag="t0")
                nc.vector.tensor_add(t0[:, :nt], ms[0][:, :nt], ms[1][:, :nt])
                t1 = mpool.tile([OC, NTMAX], BF16, name="t1", tag="t1")
                nc.vector.tensor_add(t1[:, :nt], ms[2][:, :nt], ms[3][:, :nt])
                t2 = mpool.tile([OC, NTMAX], BF16, name="t2", tag="t2")
                nc.vector.tensor_add(t2[:, :nt], t0[:, :nt], t1[:, :nt])
                ot = opool.tile([OC, NTMAX], BF16, name="ot", tag="ot")
                nc.vector.tensor_add(
                    ot[:, :nt], t2[:, :nt], bias_sb[:, p0 : p0 + nt]
                )
                RINGS[2 * (blk % 2)].dma_start(
                    out=out[img][:, p0 : p0 + nt], in_=ot[:, :nt]
                )
            if img + 1 < B_LOC:
                x_cur, lay_cur = x_nxt, XCH
    nc.finalize()
    return nc


def _prep_inputs(x, w, cw_row, cw_col, b_row, b_col, b):
    # zero-padded bf16 input
    xp = np.zeros((B, IC, HP, WP), dtype=ml_dtypes.bfloat16)
    xp[:, :, 1 : H + 1, 1 : W + 1] = x.astype(ml_dtypes.bfloat16)

    # weights: [rank, oc, ic, kh, kw] -> [ic, rank, kh*kw, oc], bf16
    wt = np.ascontiguousarray(np.transpose(w, (2, 0, 3, 4, 1))).reshape(
        IC, RANK, 9, OC
    ).astype(ml_dtypes.bfloat16)

    # softmax over rank of per-pixel combine logits
    logits = (cw_row + cw_col).astype(np.float64)  # [rank, H, W]
    logits -= logits.max(axis=0, keepdims=True)
    e = np.exp(logits)
    cw = (
        (e / e.sum(axis=0, keepdims=True))
        .astype(ml_dtypes.bfloat16)
        .reshape(RANK, PIX)
    )

    # combined bias plane [oc, pix]
    bias = (
        b.reshape(OC, 1, 1) + b_row.reshape(1, H, 1) + b_col.reshape(1, 1, W)
    ).astype(ml_dtypes.bfloat16).reshape(OC, PIX)

    return xp, wt, cw, bias


def _run(inputs, trace=False):
    if "nc" not in _CACHE:
        _CACHE["nc"] = _build_nc()
    nc = _CACHE["nc"]
    xp, wt, cw, bias = _prep_inputs(**inputs)
    in_maps = [
        {"x": xp[c * B_LOC : (c + 1) * B_LOC], "w": wt, "cw": cw, "bias": bias}
        for c in range(NCORES)
    ]
    res = run_bass_kernel_spmd(nc, in_maps, list(range(NCORES)), trace=trace)
    outs = [np.asarray(res.results[c]["out"]) for c in range(NCORES)]
    full = np.concatenate(outs, axis=0).reshape(B, OC, H, W).astype(np.float32)
    return full, res


def kernel(**inputs):
    full, _ = _run(inputs)
    return full



# revision 12
# speedup vs baseline: 1.2284x; 1.0197x over previous
"""Trainium2 Bass kernel for DSSConv2d (low-rank spatially-combined 3x3 conv).

Computation (per reference):
  convs = conv2d(x, w.reshape(rank*oc, ic, 3, 3), pad=1)   # [B, rank*oc, H, W]
  cw    = softmax(cw_row + cw_col, axis=0)                 # [rank, H, W]
  out   = einsum('bkcxy,kxy->bcxy', convs.reshape(B,rank,oc,H,W), cw)
  out  += b + b_col + b_row

Strategy:
  - Data parallel: batch 32 -> 4 images per core on 8 cores.
  - 3x3 conv = 9 shifted 1x1 convs: for each (dy,dx), a [ic,oc] matmul over a
    shifted slice of zero-padded x, accumulated in PSUM. bf16 inputs, fp32 acc.
  - Layout: psum[oc, pix] with weights stationary; pixel tiles = 8 image rows
    (N = 8*56 = 448 <= 512 PSUM bank).
  - Per-pixel rank combine on VectorE with the softmax weights broadcast
    across partitions via a stride-0 DMA; bias plane precomputed on host.
"""

import numpy as np
import ml_dtypes
from contextlib import ExitStack

import concourse.bass as bass
import concourse.mybir as mybir
import concourse.tile as tile
from concourse import bacc
from concourse.bass_utils import run_bass_kernel_spmd

RANK, OC, IC = 4, 128, 128
B, H, W = 32, 56, 56
NCORES = 8
B_LOC = B // NCORES          # 4 images per core
HP, WP = H + 2, W + 2        # zero-padded input: 58 x 58
PIX = H * W                  # 3136
RB = 8                       # output rows per pixel tile
NBLK = H // RB               # 7 tiles per image
NT = RB * W                  # 448 pixels per tile

BF16 = mybir.dt.bfloat16
F32 = mybir.dt.float32

_CACHE = {}


def _build_nc():
    nc = bacc.Bacc()
    x_in = nc.dram_tensor("x", [B_LOC, IC, HP, WP], BF16, kind="ExternalInput")
    w_in = nc.dram_tensor("w", [IC, RANK, 9, OC], BF16, kind="ExternalInput")
    cw_in = nc.dram_tensor("cw", [RANK, PIX], BF16, kind="ExternalInput")
    bias_in = nc.dram_tensor("bias", [OC, PIX], BF16, kind="ExternalInput")
    out = nc.dram_tensor("out", [B_LOC, OC, PIX], BF16, kind="ExternalOutput")

    with tile.TileContext(nc) as tc, ExitStack() as ctx:
        consts = ctx.enter_context(tc.tile_pool(name="consts", bufs=1))
        xpool = ctx.enter_context(tc.tile_pool(name="xpool", bufs=2))
        mpool = ctx.enter_context(tc.tile_pool(name="mpool", bufs=2))
        opool = ctx.enter_context(tc.tile_pool(name="opool", bufs=3))
        pspool = ctx.enter_context(tc.tile_pool(name="ps", bufs=8, space="PSUM"))

        # Row blocks per image: 8 rows -> matmul N = 448 (<=512 ISA limit).
        BLKS = [(h0, 8) for h0 in range(0, H, 8)]
        NTMAX = 8 * W

        # x in standalone row chunks per image (rows incl. 3x3 halo) so the
        # first matmuls wait on a ~0.26 MB transfer, not a full image.
        XCH = [(0, 18), (16, 18), (32, 18), (48, 10)]
        # image 0 split finer and spread across rings: first block only
        # needs padded rows [0, 10)
        XCH0 = [(0, 10), (8, 18), (24, 18), (40, 18)]

        RINGS = [nc.sync, nc.scalar, nc.gpsimd]

        def load_img(img, layout, spread=False):
            chunks = []
            for ci, (r0, nr) in enumerate(layout):
                xc = xpool.tile([IC, nr, WP], BF16, name=f"x{ci}", tag=f"x{ci}")
                eng = RINGS[ci % len(RINGS)] if spread else nc.scalar
                eng.dma_start(out=xc, in_=x_in[img][:, r0 : r0 + nr, :])
                chunks.append(xc)
            return chunks

        def chunk_for(layout, h0, nr):
            # deepest chunk that covers padded rows [h0, h0+nr+2)
            for ci in range(len(layout) - 1, -1, -1):
                r0, cnr = layout[ci]
                if r0 <= h0 and h0 + nr + 2 <= r0 + cnr:
                    return ci
            raise AssertionError((h0, nr))

        # Startup critical path. All DMAs are issued in first-use order and
        # round-robined across the three HWDGE rings so no ring develops a
        # deep queue ahead of a deadline:
        #   1. x image-0 chunk 0 (block 0's rhs) alone at the head of the
        #      scalar ring.
        #   2. every rank's weights per-tap (36 x 32 KB) -- tap (r, o) is
        #      consumed ~0.2 us apart, so round-robin delivery always stays
        #      ahead of the matmul stream (v2 stalled 11 us waiting for the
        #      monolithic 0.3 MB rank-1 weight load).
        #   3. remaining image-0 chunks.
        #   4. cw broadcasts + bias, chunked per row-block so block 0's
        #      combine waits on ~0.5 MB, not 4 MB.
        # Weights monolithic per rank on the sync ring: each slice reads
        # 2304 B contiguous per partition (good DMA line efficiency --
        # per-tap 256 B splits measured ~4x slower and stalled the PE), and
        # rank r always lands before its first matmul. x image 0 has the
        # scalar ring to itself; cw/bias go per-block on the gpsimd ring so
        # block 0's combine waits on ~0.5 MB, not 4 MB.
        x0c0 = xpool.tile([IC, XCH0[0][1], WP], BF16, name="x0", tag="x0")
        nc.scalar.dma_start(out=x0c0, in_=x_in[0][:, 0 : XCH0[0][1], :])

        # rank r's weights must land every ~1.7 us once block 0 is rolling;
        # one ring sustains only ~0.3 MB / 3 us, so spread the four loads:
        # sync gets w0 first, gpsimd w1 first, scalar w2 behind the first x
        # chunk, sync w3 second.
        w_sb = []
        for r, eng in zip(range(RANK), [nc.sync, nc.gpsimd, nc.scalar, nc.sync]):
            wr = consts.tile([IC, 9, OC], BF16, name=f"w{r}")
            eng.dma_start(out=wr, in_=w_in[:, r, :, :])
            w_sb.append(wr)

        x0 = [x0c0]
        for ci, (r0, nr) in enumerate(XCH0[1:], start=1):
            xc = xpool.tile([IC, nr, WP], BF16, name=f"x{ci}", tag=f"x{ci}")
            nc.scalar.dma_start(out=xc, in_=x_in[0][:, r0 : r0 + nr, :])
            x0.append(xc)

        cwb = [
            consts.tile([128, PIX], BF16, name=f"cw{r}") for r in range(RANK)
        ]
        bias_sb = consts.tile([OC, PIX], BF16)
        for blk in range(NBLK):
            p0, p1 = blk * NT, (blk + 1) * NT
            for r in range(RANK):
                nc.gpsimd.dma_start(
                    out=cwb[r][:, p0:p1],
                    in_=cw_in[r, p0:p1].partition_broadcast(128),
                )
            nc.gpsimd.dma_start(out=bias_sb[:, p0:p1], in_=bias_in[:, p0:p1])

        # HAM warmup: dummy matmuls from t~0.3us cover the HAM cold window
        # and the DMA preamble, so real matmuls start at full clock. The
        # warm tile comes from a DVE memset (fast; GPSIMD ucode boot used
        # to delay this by ~6us). A throwaway reader releases the PSUM
        # slot back to the pool before the main loop needs all 8 banks.
        warm = consts.tile([128, NTMAX], BF16, name="warm")
        nc.vector.memset(warm, 0.0)
        wps = pspool.tile([OC, NTMAX], F32, name="wps", tag="ps")
        for _ in range(10):
            nc.tensor.matmul(
                wps[:, :], lhsT=warm[:, :128], rhs=warm, start=True, stop=True
            )
        wsink = consts.tile([128, 1], F32, name="wsink")
        nc.vector.tensor_copy(wsink, wps[:, 0:1])

        x_cur, lay_cur = x0, XCH0
        for img in range(B_LOC):
            if img + 1 < B_LOC:
                x_nxt = load_img(img + 1, XCH)
            blks = BLKS
            if img == B_LOC - 1:
                # shorter final blocks -> shorter post-matmul DVE/DMA tail
                blks = BLKS[:-1] + [(48, 4), (52, 2), (54, 2)]
            for blk, (h0, nr) in enumerate(blks):
                ci = chunk_for(lay_cur, h0, nr)
                xc = x_cur[ci]
                hoff = h0 - lay_cur[ci][0]
                nt = nr * W
                p0 = h0 * W
                psums = []
                for r in range(RANK):
                    ps = pspool.tile([OC, NTMAX], F32, name=f"ps{r}", tag="ps")
                    for o in range(9):
                        dy, dx = o // 3, o % 3
                        nc.tensor.matmul(
                            ps[:, :nt],
                            lhsT=w_sb[r][:, o, :],
                            rhs=xc[:, hoff + dy : hoff + dy + nr, dx : dx + W],
                            start=(o == 0),
                            stop=(o == 8),
                        )
                    psums.append(ps)
                # combine: all four rank muls first (frees the PSUM banks
                # as early as possible), then a bf16 add tree (DVE 2x
                # packed mode) with the bias folded into the last add.
                ms = []
                for r in range(RANK):
                    m = mpool.tile([OC, NTMAX], BF16, name=f"m{r}", tag=f"m{r}")
                    nc.vector.tensor_mul(
                        m[:, :nt], psums[r][:, :nt], cwb[r][:, p0 : p0 + nt]
                    )
                    ms.append(m)
                t0 = mpool.tile([OC, NTMAX], BF16, name="t0", tag="t0")
                nc.vector.tensor_add(t0[:, :nt], ms[0][:, :nt], ms[1][:, :nt])
                t1 = mpool.tile([OC, NTMAX], BF16, name="t1", tag="t1")
                nc.vector.tensor_add(t1[:, :nt], ms[2][:, :nt], ms[3][:, :nt])
                t2 = mpool.tile([OC, NTMAX], BF16, name="t2", tag="t2")
                nc.vector.tensor_add(t2[:, :nt], t0[:, :nt], t1[:, :nt])
                ot = opool.tile([OC, NTMAX], BF16, name="ot", tag="ot")
                nc.vector.tensor_add(
                    ot[:, :nt], t2[:, :nt], bias_sb[:, p0 : p0 + nt]
                )
                RINGS[2 * (blk % 2)].dma_start(
                    out=out[img][:, p0 : p0 + nt], in_=ot[:, :nt]
                )
            if img + 1 < B_LOC:
                x_cur, lay_cur = x_nxt, XCH
    nc.finalize()
    return nc


def _prep_inputs(x, w, cw_row, cw_col, b_row, b_col, b):
    # zero-padded bf16 input
    xp = np.zeros((B, IC, HP, WP), dtype=ml_dtypes.bfloat16)
    xp[:, :, 1 : H + 1, 1 : W + 1] = x.astype(ml_dtypes.bfloat16)

    # weights: [rank, oc, ic, kh, kw] -> [ic, rank, kh*kw, oc], bf16
    wt = np.ascontiguousarray(np.transpose(w, (2, 0, 3, 4, 1))).reshape(
        IC, RANK, 9, OC
    ).astype(ml_dtypes.bfloat16)

    # softmax over rank of per-pixel combine logits
    logits = (cw_row + cw_col).astype(np.float64)  # [rank, H, W]
    logits -= logits.max(axis=0, keepdims=True)
    e = np.exp(logits)
    cw = (
        (e / e.sum(axis=0, keepdims=True))
        .astype(ml_dtypes.bfloat16)
        .reshape(RANK, PIX)
    )

    # combined bias plane [oc, pix]
    bias = (
        b.reshape(OC, 1, 1) + b_row.reshape(1, H, 1) + b_col.reshape(1, 1, W)
    ).astype(ml_dtypes.bfloat16).reshape(OC, PIX)

    return xp, wt, cw, bias


def _run(inputs, trace=False):
    if "nc" not in _CACHE:
        _CACHE["nc"] = _build_nc()
    nc = _CACHE["nc"]
    xp, wt, cw, bias = _prep_inputs(**inputs)
    in_maps = [
        {"x": xp[c * B_LOC : (c + 1) * B_LOC], "w": wt, "cw": cw, "bias": bias}
        for c in range(NCORES)
    ]
    res = run_bass_kernel_spmd(nc, in_maps, list(range(NCORES)), trace=trace)
    outs = [np.asarray(res.results[c]["out"]) for c in range(NCORES)]
    full = np.concatenate(outs, axis=0).reshape(B, OC, H, W).astype(np.float32)
    return full, res


def kernel(**inputs):
    full, _ = _run(inputs)
    return full



# revision 13
# speedup vs baseline: 1.2334x; 1.0040x over previous
"""Trainium2 Bass kernel for DSSConv2d (low-rank spatially-combined 3x3 conv).

Computation (per reference):
  convs = conv2d(x, w.reshape(rank*oc, ic, 3, 3), pad=1)   # [B, rank*oc, H, W]
  cw    = softmax(cw_row + cw_col, axis=0)                 # [rank, H, W]
  out   = einsum('bkcxy,kxy->bcxy', convs.reshape(B,rank,oc,H,W), cw)
  out  += b + b_col + b_row

Strategy:
  - Data parallel: batch 32 -> 4 images per core on 8 cores.
  - 3x3 conv = 9 shifted 1x1 convs: for each (dy,dx), a [ic,oc] matmul over a
    shifted slice of zero-padded x, accumulated in PSUM. bf16 inputs, fp32 acc.
  - Layout: psum[oc, pix] with weights stationary; pixel tiles = 8 image rows
    (N = 8*56 = 448 <= 512 PSUM bank).
  - Per-pixel rank combine on VectorE with the softmax weights broadcast
    across partitions via a stride-0 DMA; bias plane precomputed on host.
"""

import numpy as np
import ml_dtypes
from contextlib import ExitStack

import concourse.bass as bass
import concourse.mybir as mybir
import concourse.tile as tile
from concourse import bacc
from concourse.bass_utils import run_bass_kernel_spmd

RANK, OC, IC = 4, 128, 128
B, H, W = 32, 56, 56
NCORES = 8
B_LOC = B // NCORES          # 4 images per core
HP, WP = H + 2, W + 2        # zero-padded input: 58 x 58
PIX = H * W                  # 3136
RB = 8                       # output rows per pixel tile
NBLK = H // RB               # 7 tiles per image
NT = RB * W                  # 448 pixels per tile

BF16 = mybir.dt.bfloat16
F32 = mybir.dt.float32

_CACHE = {}


def _build_nc():
    nc = bacc.Bacc()
    x_in = nc.dram_tensor("x", [B_LOC, IC, HP, WP], BF16, kind="ExternalInput")
    w_in = nc.dram_tensor("w", [IC, RANK, 9, OC], BF16, kind="ExternalInput")
    cw_in = nc.dram_tensor("cw", [RANK, PIX], BF16, kind="ExternalInput")
    bias_in = nc.dram_tensor("bias", [OC, PIX], BF16, kind="ExternalInput")
    out = nc.dram_tensor("out", [B_LOC, OC, PIX], BF16, kind="ExternalOutput")

    with tile.TileContext(nc) as tc, ExitStack() as ctx:
        consts = ctx.enter_context(tc.tile_pool(name="consts", bufs=1))
        xpool = ctx.enter_context(tc.tile_pool(name="xpool", bufs=2))
        mpool = ctx.enter_context(tc.tile_pool(name="mpool", bufs=2))
        opool = ctx.enter_context(tc.tile_pool(name="opool", bufs=3))
        pspool = ctx.enter_context(tc.tile_pool(name="ps", bufs=8, space="PSUM"))

        # Row blocks per image: 8 rows -> matmul N = 448 (<=512 ISA limit).
        BLKS = [(h0, 8) for h0 in range(0, H, 8)]
        NTMAX = 8 * W

        # x in standalone row chunks per image (rows incl. 3x3 halo) so the
        # first matmuls wait on a ~0.26 MB transfer, not a full image.
        XCH = [(0, 18), (16, 18), (32, 18), (48, 10)]
        # image 0 split finer and spread across rings: first block only
        # needs padded rows [0, 10)
        XCH0 = [(0, 10), (8, 18), (24, 18), (40, 18)]

        RINGS = [nc.sync, nc.scalar, nc.gpsimd]

        def load_img(img, layout, spread=False):
            chunks = []
            for ci, (r0, nr) in enumerate(layout):
                xc = xpool.tile([IC, nr, WP], BF16, name=f"x{ci}", tag=f"x{ci}")
                eng = RINGS[ci % len(RINGS)] if spread else nc.scalar
                eng.dma_start(out=xc, in_=x_in[img][:, r0 : r0 + nr, :])
                chunks.append(xc)
            return chunks

        def chunk_for(layout, h0, nr):
            # deepest chunk that covers padded rows [h0, h0+nr+2)
            for ci in range(len(layout) - 1, -1, -1):
                r0, cnr = layout[ci]
                if r0 <= h0 and h0 + nr + 2 <= r0 + cnr:
                    return ci
            raise AssertionError((h0, nr))

        # Startup critical path. All DMAs are issued in first-use order and
        # round-robined across the three HWDGE rings so no ring develops a
        # deep queue ahead of a deadline:
        #   1. x image-0 chunk 0 (block 0's rhs) alone at the head of the
        #      scalar ring.
        #   2. every rank's weights per-tap (36 x 32 KB) -- tap (r, o) is
        #      consumed ~0.2 us apart, so round-robin delivery always stays
        #      ahead of the matmul stream (v2 stalled 11 us waiting for the
        #      monolithic 0.3 MB rank-1 weight load).
        #   3. remaining image-0 chunks.
        #   4. cw broadcasts + bias, chunked per row-block so block 0's
        #      combine waits on ~0.5 MB, not 4 MB.
        # Weights monolithic per rank on the sync ring: each slice reads
        # 2304 B contiguous per partition (good DMA line efficiency --
        # per-tap 256 B splits measured ~4x slower and stalled the PE), and
        # rank r always lands before its first matmul. x image 0 has the
        # scalar ring to itself; cw/bias go per-block on the gpsimd ring so
        # block 0's combine waits on ~0.5 MB, not 4 MB.
        x0c0 = xpool.tile([IC, XCH0[0][1], WP], BF16, name="x0", tag="x0")
        nc.scalar.dma_start(out=x0c0, in_=x_in[0][:, 0 : XCH0[0][1], :])

        # rank r's weights must land every ~1.7 us once block 0 is rolling;
        # one ring sustains only ~0.3 MB / 3 us, so spread the four loads:
        # sync gets w0 first, gpsimd w1 first, scalar w2 behind the first x
        # chunk, sync w3 second.
        w_sb = []
        for r, eng in zip(range(RANK), [nc.sync, nc.gpsimd, nc.scalar, nc.sync]):
            wr = consts.tile([IC, 9, OC], BF16, name=f"w{r}")
            eng.dma_start(out=wr, in_=w_in[:, r, :, :])
            w_sb.append(wr)

        x0 = [x0c0]
        for ci, (r0, nr) in enumerate(XCH0[1:], start=1):
            xc = xpool.tile([IC, nr, WP], BF16, name=f"x{ci}", tag=f"x{ci}")
            nc.scalar.dma_start(out=xc, in_=x_in[0][:, r0 : r0 + nr, :])
            x0.append(xc)

        cwb = [
            consts.tile([128, PIX], BF16, name=f"cw{r}") for r in range(RANK)
        ]
        bias_sb = consts.tile([OC, PIX], BF16)
        for blk in range(NBLK):
            p0, p1 = blk * NT, (blk + 1) * NT
            for r in range(RANK):
                nc.gpsimd.dma_start(
                    out=cwb[r][:, p0:p1],
                    in_=cw_in[r, p0:p1].partition_broadcast(128),
                )
            nc.gpsimd.dma_start(out=bias_sb[:, p0:p1], in_=bias_in[:, p0:p1])

        # HAM warmup: dummy matmuls from t~0.3us cover the HAM cold window
        # and the DMA preamble, so real matmuls start at full clock. The
        # warm tile comes from a DVE memset (fast; GPSIMD ucode boot used
        # to delay this by ~6us). A throwaway reader releases the PSUM
        # slot back to the pool before the main loop needs all 8 banks.
        warm = consts.tile([128, NTMAX], BF16, name="warm")
        nc.vector.memset(warm, 0.0)
        wps = pspool.tile([OC, NTMAX], F32, name="wps", tag="ps")
        for _ in range(10):
            nc.tensor.matmul(
                wps[:, :], lhsT=warm[:, :128], rhs=warm, start=True, stop=True
            )
        wsink = consts.tile([128, 1], F32, name="wsink")
        nc.vector.tensor_copy(wsink, wps[:, 0:1])

        x_cur, lay_cur = x0, XCH0
        for img in range(B_LOC):
            if img + 1 < B_LOC:
                x_nxt = load_img(img + 1, XCH)
            blks = BLKS
            if img == B_LOC - 1:
                # shorter final blocks -> shorter post-matmul DVE/DMA tail
                blks = BLKS[:-1] + [(48, 4), (52, 2), (54, 2)]
            for blk, (h0, nr) in enumerate(blks):
                ci = chunk_for(lay_cur, h0, nr)
                xc = x_cur[ci]
                hoff = h0 - lay_cur[ci][0]
                nt = nr * W
                p0 = h0 * W
                psums = []
                for r in range(RANK):
                    ps = pspool.tile([OC, NTMAX], F32, name=f"ps{r}", tag="ps")
                    for o in range(9):
                        dy, dx = o // 3, o % 3
                        nc.tensor.matmul(
                            ps[:, :nt],
                            lhsT=w_sb[r][:, o, :],
                            rhs=xc[:, hoff + dy : hoff + dy + nr, dx : dx + W],
                            start=(o == 0),
                            stop=(o == 8),
                        )
                    psums.append(ps)
                # combine: all four rank muls first (frees the PSUM banks
                # as early as possible), then a bf16 add tree (DVE 2x
                # packed mode) with the bias folded into the last add.
                ms = []
                for r in range(RANK):
                    m = mpool.tile([OC, NTMAX], BF16, name=f"m{r}", tag=f"m{r}")
                    nc.vector.tensor_mul(
                        m[:, :nt], psums[r][:, :nt], cwb[r][:, p0 : p0 + nt]
                    )
                    ms.append(m)
                t0 = mpool.tile([OC, NTMAX], BF16, name="t0", tag="t0")
                nc.vector.tensor_add(t0[:, :nt], ms[0][:, :nt], ms[1][:, :nt])
                t1 = mpool.tile([OC, NTMAX], BF16, name="t1", tag="t1")
                nc.vector.tensor_add(t1[:, :nt], ms[2][:, :nt], ms[3][:, :nt])
                t2 = mpool.tile([OC, NTMAX], BF16, name="t2", tag="t2")
                nc.vector.tensor_add(t2[:, :nt], t0[:, :nt], t1[:, :nt])
                ot = opool.tile([OC, NTMAX], BF16, name="ot", tag="ot")
                nc.vector.tensor_add(
                    ot[:, :nt], t2[:, :nt], bias_sb[:, p0 : p0 + nt]
                )
                nc.sync.dma_start(
                    out=out[img][:, p0 : p0 + nt], in_=ot[:, :nt]
                )
            if img + 1 < B_LOC:
                x_cur, lay_cur = x_nxt, XCH
    nc.finalize()
    return nc


def _prep_inputs(x, w, cw_row, cw_col, b_row, b_col, b):
    # zero-padded bf16 input
    xp = np.zeros((B, IC, HP, WP), dtype=ml_dtypes.bfloat16)
    xp[:, :, 1 : H + 1, 1 : W + 1] = x.astype(ml_dtypes.bfloat16)

    # weights: [rank, oc, ic, kh, kw] -> [ic, rank, kh*kw, oc], bf16
    wt = np.ascontiguousarray(np.transpose(w, (2, 0, 3, 4, 1))).reshape(
        IC, RANK, 9, OC
    ).astype(ml_dtypes.bfloat16)

    # softmax over rank of per-pixel combine logits
    logits = (cw_row + cw_col).astype(np.float64)  # [rank, H, W]
    logits -= logits.max(axis=0, keepdims=True)
    e = np.exp(logits)
    cw = (
        (e / e.sum(axis=0, keepdims=True))
        .astype(ml_dtypes.bfloat16)
        .reshape(RANK, PIX)
    )

    # combined bias plane [oc, pix]
    bias = (
        b.reshape(OC, 1, 1) + b_row.reshape(1, H, 1) + b_col.reshape(1, 1, W)
    ).astype(ml_dtypes.bfloat16).reshape(OC, PIX)

    return xp, wt, cw, bias


def _run(inputs, trace=False):
    if "nc" not in _CACHE:
        _CACHE["nc"] = _build_nc()
    nc = _CACHE["nc"]
    xp, wt, cw, bias = _prep_inputs(**inputs)
    in_maps = [
        {"x": xp[c * B_LOC : (c + 1) * B_LOC], "w": wt, "cw": cw, "bias": bias}
        for c in range(NCORES)
    ]
    res = run_bass_kernel_spmd(nc, in_maps, list(range(NCORES)), trace=trace)
    outs = [np.asarray(res.results[c]["out"]) for c in range(NCORES)]
    full = np.concatenate(outs, axis=0).reshape(B, OC, H, W).astype(np.float32)
    return full, res


def kernel(**inputs):
    full, _ = _run(inputs)
    return full

